# revision 4
# baseline (speedup 1.0000x reference)
"""GAT layer kernel for Trainium2, 8 NeuronCores (SPMD).

Strategy: edges are sorted by dst on the host and dst-tiles (128 nodes) are
assigned to cores in equal contiguous ranges. Node features are replicated.
Each core:
  Phase A: computes z = h @ W_lin.T for ALL nodes plus per-node attention
    scalars (a_src, a_dst), writing a DRAM table row per node:
    [z (128 f32) | a_src | a_dst | pad] (768 B rows, %256 for dma_gather).
  Phase B: per dst-tile slot, dma_gathers table rows by edge src (int16 idx,
    lo/hi half tables), builds per-chunk weighted one-hot S_w (dual-op
    tensor_scalar), computes edge weights w = exp(leaky_relu(a_src+a_dst))
    with the dst-side score expanded per-edge via a PE transpose of the
    one-hot, and accumulates numerator and denominator with PE matmuls in
    PSUM. Output tile = numerator * reciprocal(denominator).
Host gathers per-core output slots back into the full [N, 128] array.
"""

import os
import sys
import types
import contextlib
import numpy as np

N = 50000
E = 1600000
D = 128
P = 128
NTILES = (N + P - 1) // P            # 391
NNODES_PAD = NTILES * P              # 50048
TSLOTS = 49                          # tile-slots per core (8*49 >= 391)
SPLIT = 32768                        # int16 gather index limit
RLEN = 192                           # table row floats (768 B)
NEG_SLOPE = 0.01
PAD_DSTLOC = 200.0                   # sentinel: never matches iota 0..127
MAX_GIDX = 1024                      # dma_gather num_idxs limit (measured)

LAST_EXEC_NS = None

# ---------------------------------------------------------------- toolchain fixes


def _apply_tilefix():
    import concourse.tile as tile_mod
    from concourse._compat import not_none as nn
    from concourse.vector_clock import ScopedClock

    def _patched_drain_and_barrier(self, tick_clock, wait_clock):
        nc = self.nc
        probe = nc.sync.nop()
        wait_clock.add_sem_waits(
            probe.ins, ScopedClock({None: tick_clock.global_clock}))
        si = probe.ins.sync_info
        waits = list(si.on_wait) if si is not None and si.on_wait else []
        nn(nc.cur_bb).bb.instructions.remove(probe.ins)
        by_name = {h.name: h for h in self.sems.allocated().values()}
        for w in waits:
            h = by_name[w.ant_name]
            assert w.wait_mode == "sem-ge-imm", w.wait_mode
            nc.sync.wait_ge(h, w.wait_value)
        nc.sync.drain()
        nc.all_engine_barrier()
        assert self.sems is not None
        popped = nc._tile_sem_poison_stack.pop()
        assert popped is self._sem_poison
        nc.clear_and_free_semaphores(list(self.sems.allocated().values()))
        nc.all_engine_barrier()

    tile_mod.TileContext._drain_and_barrier = _patched_drain_and_barrier


def _legalize_waits(nc):
    """This container's walrus caps sync waits at 1 per instruction; hoist
    extras onto standalone EventSemaphore (wait) instructions."""
    import concourse.mybir as mybir
    MAXW = 1
    for f in nc.m.functions:
        for bb in f.blocks:
            insts = bb.instructions
            new_list = []
            changed = False
            for ins in list(insts):
                si = ins.sync_info
                waits = list(si.on_wait) if (si is not None and si.on_wait) else []
                if len(waits) > MAXW:
                    changed = True
                    extra, keep = waits[:-MAXW], waits[-MAXW:]
                    for j in range(0, len(extra), MAXW):
                        chunk = extra[j:j + MAXW]
                        ev = mybir.InstEventSemaphore(
                            name=f"{ins.name}-waitfix{j}", ins=[], outs=[])
                        ev.engine = ins.engine
                        ev.sync_info = mybir.SyncInfo(on_wait=chunk, on_update=[])
                        new_list.append(ev)
                    si.on_wait = keep
                new_list.append(ins)
            if changed:
                bb.instructions = new_list


def _apply_profhook():
    try:
        import antenv.axon_hooks  # noqa: F401
        return
    except ImportError:
        pass
    try:
        from trn_agent_boot.trn_boot import _ntff_profile_via_ctypes
        hook = _ntff_profile_via_ctypes('/opt/axon/libaxon_pjrt.so')
    except Exception:
        hook = None
    mod = types.ModuleType('antenv.axon_hooks')
    mod._hook = hook
    mod.get_axon_ntff_profile_hook = lambda: mod._hook
    mod.set_axon_ntff_profile_hook = lambda h: setattr(mod, '_hook', h)
    sys.modules['antenv.axon_hooks'] = mod


# ---------------------------------------------------------------- host prep


def _ceil_div(a, b):
    return -(-a // b)


def _wrap_idx(arr):
    """[n] int16 -> [128, n/16] wrapped-in-16-partitions, replicated x8."""
    a = np.asarray(arr, dtype=np.int16)
    assert a.size % 16 == 0
    w = a.reshape(-1, 16).T.copy()            # [16, n/16]
    return np.tile(w, (8, 1))                 # [128, n/16]


def _prepare(src, dst):
    """Sort edges by dst; build per-core, per-slot chunk schedules and index
    arrays. Returns (schedule, per_core_data)."""
    src = np.asarray(src).astype(np.int64)
    dst = np.asarray(dst).astype(np.int64)
    order = np.argsort(dst, kind="stable")
    s_s = src[order]
    d_s = dst[order]

    tile_start = np.searchsorted(d_s, np.arange(0, NNODES_PAD + P, P))
    # core m owns tiles [m*TSLOTS, min((m+1)*TSLOTS, NTILES))
    n_lo = np.zeros((8, TSLOTS), np.int64)
    n_hi = np.zeros((8, TSLOTS), np.int64)
    edges_lo = {}
    edges_hi = {}
    for m in range(8):
        for k in range(TSLOTS):
            t = m * TSLOTS + k
            if t >= NTILES:
                continue
            e0, e1 = tile_start[t], tile_start[t + 1]
            ss = s_s[e0:e1]
            dl = (d_s[e0:e1] - t * P)
            lo = ss < SPLIT
            edges_lo[(m, k)] = (ss[lo], dl[lo])
            edges_hi[(m, k)] = (ss[~lo] - SPLIT, dl[~lo])
            n_lo[m, k] = int(lo.sum())
            n_hi[m, k] = int((~lo).sum())

    C_lo = [max(1, _ceil_div(int(n_lo[:, k].max()), P)) for k in range(TSLOTS)]
    C_hi = [max(1, _ceil_div(int(n_hi[:, k].max()), P)) for k in range(TSLOTS)]

    per_core = []
    for m in range(8):
        idx_cols = []      # int16 wrapped cols, concatenated along axis 1
        dstloc_cols = []   # [P, C] f32 per slot
        for k in range(TSLOTS):
            parts = []
            for (edges, C) in ((edges_lo.get((m, k), (np.zeros(0, np.int64),) * 2), C_lo[k]),
                               (edges_hi.get((m, k), (np.zeros(0, np.int64),) * 2), C_hi[k])):
                ss, dl = edges
                n = C * P
                idx = np.zeros(n, np.int64)
                dlc = np.full(n, PAD_DSTLOC, np.float32)
                idx[: ss.size] = ss
                dlc[: dl.size] = dl.astype(np.float32)
                parts.append((idx, dlc, C))
            # idx arrays wrapped per sub-gather of <= MAX_GIDX
            slot_dl = []
            for idx, dlc, C in parts:
                j = 0
                while j < idx.size:
                    n_sub = min(MAX_GIDX, idx.size - j)
                    idx_cols.append(_wrap_idx(idx[j:j + n_sub]))
                    j += n_sub
                slot_dl.append(dlc.reshape(C, P).T)     # [P, C]
            dstloc_cols.append(np.concatenate(slot_dl, axis=1))
        idx_all = np.concatenate(idx_cols, axis=1).astype(np.int16)  # [P, sum]
        dstloc_all = np.concatenate(dstloc_cols, axis=1).astype(np.float32)
        # per-slot node-id list for the [a_src, a_dst] indirect gather
        nid = np.zeros((P, TSLOTS), np.int32)
        for k in range(TSLOTS):
            t = m * TSLOTS + k
            if t < NTILES:
                nid[:, k] = np.arange(t * P, t * P + P, dtype=np.int32)
        per_core.append({"idx": idx_all, "dstloc": dstloc_all, "nid": nid})

    schedule = (tuple(C_lo), tuple(C_hi))
    return schedule, per_core


# ---------------------------------------------------------------- device program

_BUILD_CACHE = {}


def _build(schedule, idx_width):
    import concourse.bass as bass
    import concourse.mybir as mybir
    import concourse.tile as tile
    from concourse import bacc, library_config

    C_lo, C_hi = schedule
    C_tot = [a + b for a, b in zip(C_lo, C_hi)]
    CMAX = max(C_tot)

    nc = bacc.Bacc("TRN2", dynamic_dma_scratch_size=131072, num_swdge_queues=2)
    f32 = mybir.dt.float32
    i16 = mybir.dt.int16
    i32 = mybir.dt.int32

    hT = nc.dram_tensor("hT", [P, NNODES_PAD], f32, kind="ExternalInput")
    embT = nc.dram_tensor("embT", [P, NNODES_PAD], f32, kind="ExternalInput")
    W_lin = nc.dram_tensor("W_lin", [P, P], f32, kind="ExternalInput")
    W_linT = nc.dram_tensor("W_linT", [P, P], f32, kind="ExternalInput")
    wfc = nc.dram_tensor("wfc", [P, 2], f32, kind="ExternalInput")      # [u1|u2] src half
    wemb = nc.dram_tensor("wemb", [P, 2], f32, kind="ExternalInput")    # emb halves
    iota_in = nc.dram_tensor("iota_in", [P, P], f32, kind="ExternalInput")
    ident_in = nc.dram_tensor("ident_in", [P, P], f32, kind="ExternalInput")
    idx_in = nc.dram_tensor("idx_in", [P, idx_width], i16, kind="ExternalInput")
    dstloc_in = nc.dram_tensor("dstloc_in", [P, sum(C_tot)], f32, kind="ExternalInput")
    nid_in = nc.dram_tensor("nid_in", [P, TSLOTS], i32, kind="ExternalInput")

    table = nc.dram_tensor("table", [NNODES_PAD, RLEN], f32)
    out_d = nc.dram_tensor("out", [TSLOTS * P, P], f32, kind="ExternalOutput")

    with tile.TileContext(nc) as tc:
        with tc.tile_pool(name="const", bufs=1) as cpool:
            nc.gpsimd.load_library(library_config.mlp)
            iota_t = cpool.tile([P, P], f32)
            ident_t = cpool.tile([P, P], f32)
            ones_t = cpool.tile([P, 1], f32)
            rhs1 = cpool.tile([P, P], f32)          # W_linT
            u_t = cpool.tile([P, 2], f32)           # [u1|u2]
            wemb_t = cpool.tile([P, 2], f32)
            wl_t = cpool.tile([P, P], f32)
            wfc_t = cpool.tile([P, 2], f32)
            nc.sync.dma_start(out=iota_t[:], in_=iota_in[:])
            nc.sync.dma_start(out=ident_t[:], in_=ident_in[:])
            nc.sync.dma_start(out=rhs1[:], in_=W_linT[:])
            nc.sync.dma_start(out=wemb_t[:], in_=wemb[:])
            nc.sync.dma_start(out=wl_t[:], in_=W_lin[:])
            nc.sync.dma_start(out=wfc_t[:], in_=wfc[:])
            nc.vector.memset(ones_t[:], 1.0)

            # ---- u1/u2 = W_lin.T @ w_fc halves
            with tc.tile_pool(name="upsum", bufs=1, space="PSUM") as upp:
                ups = upp.tile([P, 2], f32, space="PSUM")
                nc.tensor.matmul(ups[:], lhsT=wl_t[:], rhs=wfc_t[:],
                                 start=True, stop=True)
                nc.vector.tensor_copy(out=u_t[:], in_=ups[:])

            # ---- Phase A: build node table
            with tc.tile_pool(name="pa", bufs=3) as pa, \
                 tc.tile_pool(name="pap", bufs=4, space="PSUM") as pap:
                for t in range(NTILES):
                    hT_t = pa.tile([P, P], f32, tag="hT_t")
                    embT_t = pa.tile([P, P], f32, tag="embT_t")
                    nc.sync.dma_start(out=hT_t[:], in_=hT[:, t * P:(t + 1) * P])
                    nc.sync.dma_start(out=embT_t[:], in_=embT[:, t * P:(t + 1) * P])
                    ps_z = pap.tile([P, P], f32, space="PSUM", tag="ps_z")
                    ps_s = pap.tile([P, 2], f32, space="PSUM", tag="ps_s")
                    nc.tensor.matmul(ps_z[:], lhsT=hT_t[:], rhs=rhs1[:],
                                     start=True, stop=True)
                    nc.tensor.matmul(ps_s[:], lhsT=hT_t[:], rhs=u_t[:],
                                     start=True, stop=False)
                    nc.tensor.matmul(ps_s[:], lhsT=embT_t[:], rhs=wemb_t[:],
                                     start=False, stop=True)
                    row_t = pa.tile([P, 130], f32, tag="row_t")
                    nc.scalar.copy(out=row_t[:, 0:128], in_=ps_z[:])
                    nc.vector.tensor_copy(out=row_t[:, 128:130], in_=ps_s[:])
                    nc.sync.dma_start(
                        out=table[t * P:(t + 1) * P, 0:130], in_=row_t[:])

            # ---- Phase B: per dst-tile slot
            with tc.tile_pool(name="pb", bufs=2) as pb, \
                 tc.tile_pool(name="pbs", bufs=3) as pbs, \
                 tc.tile_pool(name="pbp", bufs=2, space="PSUM") as pbp, \
                 tc.tile_pool(name="pbp2", bufs=2, space="PSUM") as pbp2:
                idx_off = 0
                dl_off = 0
                gq = 0
                for k in range(TSLOTS):
                    C = C_tot[k]
                    gbuf = pb.tile([P, C, RLEN], f32, tag="gbuf")
                    # gathers (lo then hi), each <= MAX_GIDX idxs
                    for (Ch, base) in ((C_lo[k], 0), (C_hi[k], SPLIT)):
                        ntot = Ch * P
                        cpos = 0 if base == 0 else C_lo[k]
                        j = 0
                        while j < ntot:
                            n_sub = min(MAX_GIDX, ntot - j)
                            w16 = n_sub // 16
                            it = pbs.tile([P, w16], i16, tag="idx")
                            nc.sync.dma_start(
                                out=it[:], in_=idx_in[:, idx_off:idx_off + w16])
                            idx_off += w16
                            c0 = cpos + j // P
                            nsc = n_sub // P
                            if base == 0:
                                src_ap = table[0:SPLIT]
                            else:
                                src_ap = table[SPLIT:NNODES_PAD]
                            nc.gpsimd.dma_gather(
                                out_ap=gbuf[:, c0:c0 + nsc, :], in_ap=src_ap,
                                idxs_ap=it[:], num_idxs=n_sub,
                                num_idxs_reg=n_sub, elem_size=RLEN,
                                queue_num=gq % 2)
                            gq += 1
                            j += n_sub
                    # per-slot [a_src|a_dst] of the 128 dst nodes
                    nidt = pbs.tile([P, 1], i32, tag="nidt")
                    nc.sync.dma_start(out=nidt[:], in_=nid_in[:, k:k + 1])
                    ad_t = pbs.tile([P, 2], f32, tag="ad_t")
                    nc.gpsimd.indirect_dma_start(
                        out=ad_t[:], out_offset=None,
                        in_=table[:],
                        in_offset=bass.IndirectOffsetOnAxis(ap=nidt[:, 0:1], axis=0),
                        element_offset=128)
                    # dst_local columns for this slot
                    dl_t = pbs.tile([P, C], f32, tag="dl_t")
                    nc.sync.dma_start(
                        out=dl_t[:], in_=dstloc_in[:, dl_off:dl_off + C])
                    dl_off += C

                    # a_dst expanded per edge: per chunk via PE transpose
                    ps_adst = pbp2.tile([P, C], f32, space="PSUM", tag="ps_adst")
                    for c in range(C):
                        O_t = pbs.tile([P, P], f32, tag="O_t")
                        nc.vector.tensor_scalar(
                            out=O_t[:], in0=iota_t[:],
                            scalar1=dl_t[:, c:c + 1], scalar2=None,
                            op0=mybir.AluOpType.is_equal)
                        ps_T = pbp.tile([P, P], f32, space="PSUM", tag="ps_T")
                        nc.tensor.transpose(
                            out=ps_T[:], in_=O_t[:], identity=ident_t[:])
                        OT_s = pbs.tile([P, P], f32, tag="OT_s")
                        nc.scalar.copy(out=OT_s[:], in_=ps_T[:])
                        nc.tensor.matmul(
                            ps_adst[:, c:c + 1], lhsT=OT_s[:],
                            rhs=ad_t[:, 1:2], start=True, stop=True)
                    # edge scores -> weights
                    s_sl = pbs.tile([P, C], f32, tag="s_sl")
                    nc.vector.tensor_tensor(
                        out=s_sl[:],
                        in0=gbuf[:, :, 128:129].rearrange("p c one -> p (c one)"),
                        in1=ps_adst[:], op=mybir.AluOpType.add)
                    # leaky_relu(s) = max(s,0) + NEG_SLOPE*min(s,0)
                    pos_sl = pbs.tile([P, C], f32, tag="pos_sl")
                    nc.vector.tensor_scalar(
                        out=pos_sl[:], in0=s_sl[:], scalar1=0.0, scalar2=None,
                        op0=mybir.AluOpType.max)
                    negs_sl = pbs.tile([P, C], f32, tag="negs_sl")
                    nc.vector.tensor_scalar(
                        out=negs_sl[:], in0=s_sl[:], scalar1=0.0,
                        scalar2=NEG_SLOPE, op0=mybir.AluOpType.min,
                        op1=mybir.AluOpType.mult)
                    lr_sl = pbs.tile([P, C], f32, tag="lr_sl")
                    nc.vector.tensor_tensor(
                        out=lr_sl[:], in0=pos_sl[:], in1=negs_sl[:],
                        op=mybir.AluOpType.add)
                    w_sl = pbs.tile([P, C], f32, tag="w_sl")
                    nc.scalar.activation(
                        out=w_sl[:], in_=lr_sl[:],
                        func=mybir.ActivationFunctionType.Exp)
                    # weighted one-hot matmuls
                    ps_num = pbp.tile([P, P], f32, space="PSUM", tag="ps_num")
                    ps_den = pbp2.tile([P, 1], f32, space="PSUM", tag="ps_den")
                    for c in range(C):
                        S_w = pbs.tile([P, P], f32, tag="S_w")
                        nc.vector.tensor_scalar(
                            out=S_w[:], in0=iota_t[:],
                            scalar1=dl_t[:, c:c + 1], scalar2=w_sl[:, c:c + 1],
                            op0=mybir.AluOpType.is_equal,
                            op1=mybir.AluOpType.mult)
                        nc.tensor.matmul(ps_num[:], lhsT=S_w[:],
                                         rhs=gbuf[:, c, 0:128],
                                         start=(c == 0), stop=(c == C - 1))
                        nc.tensor.matmul(ps_den[:], lhsT=S_w[:], rhs=ones_t[:],
                                         start=(c == 0), stop=(c == C - 1))
                    den_s = pbs.tile([P, 1], f32, tag="den_s")
                    nc.vector.tensor_scalar(
                        out=den_s[:], in0=ps_den[:], scalar1=1e-30, scalar2=None,
                        op0=mybir.AluOpType.add)
                    den_r = pbs.tile([P, 1], f32, tag="den_r")
                    nc.vector.reciprocal(out=den_r[:], in_=den_s[:])
                    o_t = pbs.tile([P, P], f32, tag="o_t")
                    nc.vector.tensor_scalar(
                        out=o_t[:], in0=ps_num[:], scalar1=den_r[:, 0:1],
                        scalar2=None, op0=mybir.AluOpType.mult)
                    nc.sync.dma_start(
                        out=out_d[k * P:(k + 1) * P, :], in_=o_t[:])

    nc.compile()
    _legalize_waits(nc)
    return nc


# ---------------------------------------------------------------- entry point


def kernel(h, embedding, W_lin, w_fc, w_emb, src, dst):
    global LAST_EXEC_NS
    _apply_tilefix()
    _apply_profhook()
    from concourse import bass_utils

    h = np.asarray(h, dtype=np.float32)
    embedding = np.asarray(embedding, dtype=np.float32)
    W_lin = np.asarray(W_lin, dtype=np.float32)
    w_fc = np.asarray(w_fc, dtype=np.float32).reshape(-1)
    w_emb = np.asarray(w_emb, dtype=np.float32).reshape(-1)

    schedule, per_core = _prepare(src, dst)
    idx_width = per_core[0]["idx"].shape[1]
    key = (schedule, idx_width)
    if key not in _BUILD_CACHE:
        _BUILD_CACHE[key] = _build(schedule, idx_width)
    nc = _BUILD_CACHE[key]

    hT_np = np.zeros((P, NNODES_PAD), np.float32)
    hT_np[:, :N] = h.T
    embT_np = np.zeros((P, NNODES_PAD), np.float32)
    embT_np[:, :N] = embedding.T
    iota_np = np.tile(np.arange(P, dtype=np.float32)[None, :], (P, 1))
    ident_np = np.eye(P, dtype=np.float32)
    wfc_np = np.stack([w_fc[:D], w_fc[D:]], axis=1).astype(np.float32)
    wemb_np = np.stack([w_emb[:D], w_emb[D:]], axis=1).astype(np.float32)

    in_maps = []
    for m in range(8):
        in_maps.append({
            "hT": hT_np, "embT": embT_np,
            "W_lin": W_lin, "W_linT": W_lin.T.copy(),
            "wfc": wfc_np, "wemb": wemb_np,
            "iota_in": iota_np, "ident_in": ident_np,
            "idx_in": per_core[m]["idx"],
            "dstloc_in": per_core[m]["dstloc"],
            "nid_in": per_core[m]["nid"],
        })

    trace = os.environ.get("GAT_TRACE", "0") == "1"
    res = bass_utils.run_bass_kernel_spmd(
        nc, in_maps, core_ids=list(range(8)), trace=trace)
    LAST_EXEC_NS = res.exec_time_ns

    out = np.zeros((NNODES_PAD, P), np.float32)
    for m in range(8):
        t0 = m * TSLOTS
        nt = min(TSLOTS, NTILES - t0)
        if nt <= 0:
            continue
        out[t0 * P:(t0 + nt) * P] = res.results[m]["out"][: nt * P]
    return out[:N]


# revision 7
# speedup vs baseline: 1.5249x; 1.5249x over previous
"""GAT layer kernel for Trainium2, 8 NeuronCores (SPMD).

Strategy: edges are sorted by dst on the host and dst-tiles (128 nodes) are
assigned to cores in equal contiguous ranges. Node features are replicated.
Each core:
  Phase A: computes z = h @ W_lin.T for ALL nodes plus per-node attention
    scalars (a_src, a_dst), writing a DRAM table row per node:
    [z (128 f32) | a_src | a_dst | pad] (768 B rows, %256 for dma_gather).
  Phase B: per dst-tile slot, dma_gathers table rows by edge src (int16 idx,
    lo/hi half tables), builds per-chunk weighted one-hot S_w (dual-op
    tensor_scalar), computes edge weights w = exp(leaky_relu(a_src+a_dst))
    with the dst-side score expanded per-edge via a PE transpose of the
    one-hot, and accumulates numerator and denominator with PE matmuls in
    PSUM. Output tile = numerator * reciprocal(denominator).
Host gathers per-core output slots back into the full [N, 128] array.
"""

import os
import sys
import types
import contextlib
import numpy as np

N = 50000
E = 1600000
D = 128
P = 128
NTILES = (N + P - 1) // P            # 391
NNODES_PAD = NTILES * P              # 50048
TSLOTS = 49                          # tile-slots per core (8*49 >= 391)
SPLIT = 32768                        # int16 gather index limit
RLEN = 256                           # table row bf16 elems (512 B)
NEG_SLOPE = 0.01
PAD_DSTLOC = 200.0                   # sentinel: never matches iota 0..127
MAX_GIDX = 1024                      # dma_gather num_idxs limit (measured)

LAST_EXEC_NS = None

# ---------------------------------------------------------------- toolchain fixes


def _apply_tilefix():
    import concourse.tile as tile_mod
    from concourse._compat import not_none as nn
    from concourse.vector_clock import ScopedClock

    def _patched_drain_and_barrier(self, tick_clock, wait_clock):
        nc = self.nc
        probe = nc.sync.nop()
        wait_clock.add_sem_waits(
            probe.ins, ScopedClock({None: tick_clock.global_clock}))
        si = probe.ins.sync_info
        waits = list(si.on_wait) if si is not None and si.on_wait else []
        nn(nc.cur_bb).bb.instructions.remove(probe.ins)
        by_name = {h.name: h for h in self.sems.allocated().values()}
        for w in waits:
            h = by_name[w.ant_name]
            assert w.wait_mode == "sem-ge-imm", w.wait_mode
            nc.sync.wait_ge(h, w.wait_value)
        nc.sync.drain()
        nc.all_engine_barrier()
        assert self.sems is not None
        popped = nc._tile_sem_poison_stack.pop()
        assert popped is self._sem_poison
        nc.clear_and_free_semaphores(list(self.sems.allocated().values()))
        nc.all_engine_barrier()

    tile_mod.TileContext._drain_and_barrier = _patched_drain_and_barrier


def _legalize_waits(nc):
    """This container's walrus caps sync waits at 1 per instruction; hoist
    extras onto standalone EventSemaphore (wait) instructions."""
    import concourse.mybir as mybir
    MAXW = 1
    for f in nc.m.functions:
        for bb in f.blocks:
            insts = bb.instructions
            new_list = []
            changed = False
            for ins in list(insts):
                si = ins.sync_info
                waits = list(si.on_wait) if (si is not None and si.on_wait) else []
                if len(waits) > MAXW:
                    changed = True
                    extra, keep = waits[:-MAXW], waits[-MAXW:]
                    for j in range(0, len(extra), MAXW):
                        chunk = extra[j:j + MAXW]
                        ev = mybir.InstEventSemaphore(
                            name=f"{ins.name}-waitfix{j}", ins=[], outs=[])
                        ev.engine = ins.engine
                        ev.sync_info = mybir.SyncInfo(on_wait=chunk, on_update=[])
                        new_list.append(ev)
                    si.on_wait = keep
                new_list.append(ins)
            if changed:
                bb.instructions = new_list


def _apply_profhook():
    try:
        import antenv.axon_hooks  # noqa: F401
        return
    except ImportError:
        pass
    try:
        from trn_agent_boot.trn_boot import _ntff_profile_via_ctypes
        hook = _ntff_profile_via_ctypes('/opt/axon/libaxon_pjrt.so')
    except Exception:
        hook = None
    mod = types.ModuleType('antenv.axon_hooks')
    mod._hook = hook
    mod.get_axon_ntff_profile_hook = lambda: mod._hook
    mod.set_axon_ntff_profile_hook = lambda h: setattr(mod, '_hook', h)
    sys.modules['antenv.axon_hooks'] = mod


# ---------------------------------------------------------------- host prep


def _ceil_div(a, b):
    return -(-a // b)


def _wrap_idx(arr):
    """[n] int16 -> [128, n/16] wrapped-in-16-partitions, replicated x8."""
    a = np.asarray(arr, dtype=np.int16)
    assert a.size % 16 == 0
    w = a.reshape(-1, 16).T.copy()            # [16, n/16]
    return np.tile(w, (8, 1))                 # [128, n/16]


def _prepare(src, dst):
    """Sort edges by dst; build per-core, per-slot chunk schedules and index
    arrays. Returns (schedule, per_core_data)."""
    src = np.asarray(src).astype(np.int64)
    dst = np.asarray(dst).astype(np.int64)
    order = np.argsort(dst, kind="stable")
    s_s = src[order]
    d_s = dst[order]

    tile_start = np.searchsorted(d_s, np.arange(0, NNODES_PAD + P, P))
    # core m owns tiles [m*TSLOTS, min((m+1)*TSLOTS, NTILES))
    n_lo = np.zeros((8, TSLOTS), np.int64)
    n_hi = np.zeros((8, TSLOTS), np.int64)
    edges_lo = {}
    edges_hi = {}
    for m in range(8):
        for k in range(TSLOTS):
            t = m * TSLOTS + k
            if t >= NTILES:
                continue
            e0, e1 = tile_start[t], tile_start[t + 1]
            ss = s_s[e0:e1]
            dl = (d_s[e0:e1] - t * P)
            lo = ss < SPLIT
            edges_lo[(m, k)] = (ss[lo], dl[lo])
            edges_hi[(m, k)] = (ss[~lo] - SPLIT, dl[~lo])
            n_lo[m, k] = int(lo.sum())
            n_hi[m, k] = int((~lo).sum())

    C_lo = [max(1, _ceil_div(int(n_lo[:, k].max()), P)) for k in range(TSLOTS)]
    C_hi = [max(1, _ceil_div(int(n_hi[:, k].max()), P)) for k in range(TSLOTS)]

    per_core = []
    for m in range(8):
        idx_cols = []      # int16 wrapped cols, concatenated along axis 1
        dstloc_cols = []   # [P, C] f32 per slot
        for k in range(TSLOTS):
            parts = []
            for (edges, C) in ((edges_lo.get((m, k), (np.zeros(0, np.int64),) * 2), C_lo[k]),
                               (edges_hi.get((m, k), (np.zeros(0, np.int64),) * 2), C_hi[k])):
                ss, dl = edges
                n = C * P
                idx = np.zeros(n, np.int64)
                dlc = np.full(n, PAD_DSTLOC, np.float32)
                idx[: ss.size] = ss
                dlc[: dl.size] = dl.astype(np.float32)
                parts.append((idx, dlc, C))
            # idx arrays wrapped per sub-gather of <= MAX_GIDX
            slot_dl = []
            for idx, dlc, C in parts:
                j = 0
                while j < idx.size:
                    n_sub = min(MAX_GIDX, idx.size - j)
                    idx_cols.append(_wrap_idx(idx[j:j + n_sub]))
                    j += n_sub
                slot_dl.append(dlc.reshape(C, P).T)     # [P, C]
            dstloc_cols.append(np.concatenate(slot_dl, axis=1))
        idx_all = np.concatenate(idx_cols, axis=1).astype(np.int16)  # [P, sum]
        dstloc_all = np.concatenate(dstloc_cols, axis=1).astype(np.float32)
        # per-slot node-id list for the [a_src, a_dst] indirect gather
        nid = np.zeros((P, TSLOTS), np.int32)
        for k in range(TSLOTS):
            t = m * TSLOTS + k
            if t < NTILES:
                nid[:, k] = np.arange(t * P, t * P + P, dtype=np.int32)
        per_core.append({"idx": idx_all, "dstloc": dstloc_all, "nid": nid})

    schedule = (tuple(C_lo), tuple(C_hi))
    return schedule, per_core


# ---------------------------------------------------------------- device program

_BUILD_CACHE = {}


def _build(schedule, idx_width):
    import concourse.bass as bass
    import concourse.mybir as mybir
    import concourse.tile as tile
    from concourse import bacc, library_config

    C_lo, C_hi = schedule
    C_tot = [a + b for a, b in zip(C_lo, C_hi)]
    CMAX = max(C_tot)

    nc = bacc.Bacc("TRN2", dynamic_dma_scratch_size=131072, num_swdge_queues=2)
    f32 = mybir.dt.float32
    i16 = mybir.dt.int16
    i32 = mybir.dt.int32

    hT = nc.dram_tensor("hT", [P, NNODES_PAD], f32, kind="ExternalInput")
    embT = nc.dram_tensor("embT", [P, NNODES_PAD], f32, kind="ExternalInput")
    W_lin = nc.dram_tensor("W_lin", [P, P], f32, kind="ExternalInput")
    W_linT = nc.dram_tensor("W_linT", [P, P], f32, kind="ExternalInput")
    wfc = nc.dram_tensor("wfc", [P, 2], f32, kind="ExternalInput")      # [u1|u2] src half
    wemb = nc.dram_tensor("wemb", [P, 2], f32, kind="ExternalInput")    # emb halves
    iota_in = nc.dram_tensor("iota_in", [P, P], mybir.dt.bfloat16, kind="ExternalInput")
    ident_in = nc.dram_tensor("ident_in", [P, P], mybir.dt.bfloat16, kind="ExternalInput")
    idx_in = nc.dram_tensor("idx_in", [P, idx_width], i16, kind="ExternalInput")
    dstloc_in = nc.dram_tensor("dstloc_in", [P, sum(C_tot)], f32, kind="ExternalInput")
    nid_in = nc.dram_tensor("nid_in", [P, TSLOTS], i32, kind="ExternalInput")

    bf16 = mybir.dt.bfloat16
    table = nc.dram_tensor("table", [NNODES_PAD, RLEN], bf16)
    out_d = nc.dram_tensor("out", [TSLOTS * P, P], f32, kind="ExternalOutput")

    with tile.TileContext(nc) as tc:
        with tc.tile_pool(name="const", bufs=1) as cpool:
            nc.gpsimd.load_library(library_config.mlp)
            iota_t = cpool.tile([P, P], bf16)
            ident_t = cpool.tile([P, P], bf16)
            ones_t = cpool.tile([P, 1], bf16)
            rhs1 = cpool.tile([P, P], f32)          # W_linT
            u_t = cpool.tile([P, 2], f32)           # [u1|u2]
            wemb_t = cpool.tile([P, 2], f32)
            wl_t = cpool.tile([P, P], f32)
            wfc_t = cpool.tile([P, 2], f32)
            nc.sync.dma_start(out=iota_t[:], in_=iota_in[:])
            nc.sync.dma_start(out=ident_t[:], in_=ident_in[:])
            nc.sync.dma_start(out=rhs1[:], in_=W_linT[:])
            nc.sync.dma_start(out=wemb_t[:], in_=wemb[:])
            nc.sync.dma_start(out=wl_t[:], in_=W_lin[:])
            nc.sync.dma_start(out=wfc_t[:], in_=wfc[:])
            nc.vector.memset(ones_t[:], 1.0)

            # ---- u1/u2 = W_lin.T @ w_fc halves
            with tc.tile_pool(name="upsum", bufs=1, space="PSUM") as upp:
                ups = upp.tile([P, 2], f32, space="PSUM")
                nc.tensor.matmul(ups[:], lhsT=wl_t[:], rhs=wfc_t[:],
                                 start=True, stop=True)
                nc.vector.tensor_copy(out=u_t[:], in_=ups[:])

            # ---- Phase A: build node table
            with tc.tile_pool(name="pa", bufs=3) as pa, \
                 tc.tile_pool(name="pap", bufs=4, space="PSUM") as pap:
                for t in range(NTILES):
                    hT_t = pa.tile([P, P], f32, tag="hT_t")
                    embT_t = pa.tile([P, P], f32, tag="embT_t")
                    nc.sync.dma_start(out=hT_t[:], in_=hT[:, t * P:(t + 1) * P])
                    nc.sync.dma_start(out=embT_t[:], in_=embT[:, t * P:(t + 1) * P])
                    ps_z = pap.tile([P, P], f32, space="PSUM", tag="ps_z")
                    ps_s = pap.tile([P, 2], f32, space="PSUM", tag="ps_s")
                    nc.tensor.matmul(ps_z[:], lhsT=hT_t[:], rhs=rhs1[:],
                                     start=True, stop=True)
                    nc.tensor.matmul(ps_s[:], lhsT=hT_t[:], rhs=u_t[:],
                                     start=True, stop=False)
                    nc.tensor.matmul(ps_s[:], lhsT=embT_t[:], rhs=wemb_t[:],
                                     start=False, stop=True)
                    row_t = pa.tile([P, 132], bf16, tag="row_t")
                    nc.scalar.copy(out=row_t[:, 0:128], in_=ps_z[:])
                    nc.vector.tensor_copy(
                        out=row_t[:, 128:132].bitcast(f32), in_=ps_s[:])
                    nc.sync.dma_start(
                        out=table[t * P:(t + 1) * P, 0:132], in_=row_t[:])

            # ---- Phase B: per dst-tile slot
            with tc.tile_pool(name="pb", bufs=2) as pb, \
                 tc.tile_pool(name="pbs", bufs=3) as pbs, \
                 tc.tile_pool(name="pbp", bufs=2, space="PSUM") as pbp, \
                 tc.tile_pool(name="pbp2", bufs=2, space="PSUM") as pbp2:
                idx_off = 0
                dl_off = 0
                gq = 0
                for k in range(TSLOTS):
                    C = C_tot[k]
                    gbuf = pb.tile([P, C, RLEN], bf16, tag="gbuf")
                    # gathers (lo then hi), each <= MAX_GIDX idxs
                    for (Ch, base) in ((C_lo[k], 0), (C_hi[k], SPLIT)):
                        ntot = Ch * P
                        cpos = 0 if base == 0 else C_lo[k]
                        j = 0
                        while j < ntot:
                            n_sub = min(MAX_GIDX, ntot - j)
                            w16 = n_sub // 16
                            it = pbs.tile([P, w16], i16, tag="idx")
                            nc.sync.dma_start(
                                out=it[:], in_=idx_in[:, idx_off:idx_off + w16])
                            idx_off += w16
                            c0 = cpos + j // P
                            nsc = n_sub // P
                            if base == 0:
                                src_ap = table[0:SPLIT]
                            else:
                                src_ap = table[SPLIT:NNODES_PAD]
                            nc.gpsimd.dma_gather(
                                out_ap=gbuf[:, c0:c0 + nsc, :], in_ap=src_ap,
                                idxs_ap=it[:], num_idxs=n_sub,
                                num_idxs_reg=n_sub, elem_size=RLEN,
                                queue_num=gq % 2)
                            gq += 1
                            j += n_sub
                    # per-slot [a_src|a_dst] of the 128 dst nodes
                    nidt = pbs.tile([P, 1], i32, tag="nidt")
                    nc.sync.dma_start(out=nidt[:], in_=nid_in[:, k:k + 1])
                    ad_bt = pbs.tile([P, 4], bf16, tag="ad_bt")
                    nc.gpsimd.indirect_dma_start(
                        out=ad_bt[:], out_offset=None,
                        in_=table[:],
                        in_offset=bass.IndirectOffsetOnAxis(ap=nidt[:, 0:1], axis=0),
                        element_offset=128)
                    ad_t = ad_bt[:, 0:4].bitcast(f32)
                    # dst_local columns for this slot
                    dl_t = pbs.tile([P, C], f32, tag="dl_t")
                    nc.sync.dma_start(
                        out=dl_t[:], in_=dstloc_in[:, dl_off:dl_off + C])
                    dl_off += C

                    # a_dst expanded per edge: per chunk via PE transpose
                    ps_adst = pbp2.tile([P, C], f32, space="PSUM", tag="ps_adst")
                    for c in range(C):
                        O_t = pbs.tile([P, P], bf16, tag="O_t")
                        nc.vector.tensor_scalar(
                            out=O_t[:], in0=iota_t[:],
                            scalar1=dl_t[:, c:c + 1], scalar2=None,
                            op0=mybir.AluOpType.is_equal)
                        ps_T = pbp.tile([P, P], bf16, space="PSUM", tag="ps_T")
                        nc.tensor.transpose(
                            out=ps_T[:], in_=O_t[:], identity=ident_t[:])
                        OT_s = pbs.tile([P, P], f32, tag="OT_s")
                        nc.scalar.copy(out=OT_s[:], in_=ps_T[:])
                        nc.tensor.matmul(
                            ps_adst[:, c:c + 1], lhsT=OT_s[:],
                            rhs=ad_t[:, 1:2], start=True, stop=True)
                    # edge scores -> weights
                    s_sl = pbs.tile([P, C], f32, tag="s_sl")
                    nc.vector.tensor_tensor(
                        out=s_sl[:],
                        in0=gbuf[:, :, 128:132].bitcast(f32)[:, :, 0:1]
                            .rearrange("p c one -> p (c one)"),
                        in1=ps_adst[:], op=mybir.AluOpType.add)
                    # leaky_relu(s) = max(s,0) + NEG_SLOPE*min(s,0)
                    pos_sl = pbs.tile([P, C], f32, tag="pos_sl")
                    nc.vector.tensor_scalar(
                        out=pos_sl[:], in0=s_sl[:], scalar1=0.0, scalar2=None,
                        op0=mybir.AluOpType.max)
                    negs_sl = pbs.tile([P, C], f32, tag="negs_sl")
                    nc.vector.tensor_scalar(
                        out=negs_sl[:], in0=s_sl[:], scalar1=0.0,
                        scalar2=NEG_SLOPE, op0=mybir.AluOpType.min,
                        op1=mybir.AluOpType.mult)
                    lr_sl = pbs.tile([P, C], f32, tag="lr_sl")
                    nc.vector.tensor_tensor(
                        out=lr_sl[:], in0=pos_sl[:], in1=negs_sl[:],
                        op=mybir.AluOpType.add)
                    w_sl = pbs.tile([P, C], f32, tag="w_sl")
                    nc.scalar.activation(
                        out=w_sl[:], in_=lr_sl[:],
                        func=mybir.ActivationFunctionType.Exp)
                    # weighted one-hot matmuls
                    ps_num = pbp.tile([P, P], f32, space="PSUM", tag="ps_num")
                    ps_den = pbp2.tile([P, 1], f32, space="PSUM", tag="ps_den")
                    for c in range(C):
                        S_w = pbs.tile([P, P], bf16, tag="S_w")
                        nc.vector.tensor_scalar(
                            out=S_w[:], in0=iota_t[:],
                            scalar1=dl_t[:, c:c + 1], scalar2=w_sl[:, c:c + 1],
                            op0=mybir.AluOpType.is_equal,
                            op1=mybir.AluOpType.mult)
                        nc.tensor.matmul(ps_num[:], lhsT=S_w[:],
                                         rhs=gbuf[:, c, 0:128],
                                         start=(c == 0), stop=(c == C - 1))
                        nc.tensor.matmul(ps_den[:], lhsT=S_w[:], rhs=ones_t[:],
                                         start=(c == 0), stop=(c == C - 1))
                    den_s = pbs.tile([P, 1], f32, tag="den_s")
                    nc.vector.tensor_scalar(
                        out=den_s[:], in0=ps_den[:], scalar1=1e-30, scalar2=None,
                        op0=mybir.AluOpType.add)
                    den_r = pbs.tile([P, 1], f32, tag="den_r")
                    nc.vector.reciprocal(out=den_r[:], in_=den_s[:])
                    o_t = pbs.tile([P, P], f32, tag="o_t")
                    nc.vector.tensor_scalar(
                        out=o_t[:], in0=ps_num[:], scalar1=den_r[:, 0:1],
                        scalar2=None, op0=mybir.AluOpType.mult)
                    nc.sync.dma_start(
                        out=out_d[k * P:(k + 1) * P, :], in_=o_t[:])

    nc.compile()
    _legalize_waits(nc)
    return nc


# ---------------------------------------------------------------- entry point


def kernel(h, embedding, W_lin, w_fc, w_emb, src, dst):
    global LAST_EXEC_NS
    _apply_tilefix()
    _apply_profhook()
    from concourse import bass_utils

    h = np.asarray(h, dtype=np.float32)
    embedding = np.asarray(embedding, dtype=np.float32)
    W_lin = np.asarray(W_lin, dtype=np.float32)
    w_fc = np.asarray(w_fc, dtype=np.float32).reshape(-1)
    w_emb = np.asarray(w_emb, dtype=np.float32).reshape(-1)

    schedule, per_core = _prepare(src, dst)
    idx_width = per_core[0]["idx"].shape[1]
    key = (schedule, idx_width)
    if key not in _BUILD_CACHE:
        _BUILD_CACHE[key] = _build(schedule, idx_width)
    nc = _BUILD_CACHE[key]

    hT_np = np.zeros((P, NNODES_PAD), np.float32)
    hT_np[:, :N] = h.T
    embT_np = np.zeros((P, NNODES_PAD), np.float32)
    embT_np[:, :N] = embedding.T
    import ml_dtypes
    iota_np = np.tile(np.arange(P)[None, :], (P, 1)).astype(ml_dtypes.bfloat16)
    ident_np = np.eye(P).astype(ml_dtypes.bfloat16)
    wfc_np = np.stack([w_fc[:D], w_fc[D:]], axis=1).astype(np.float32)
    wemb_np = np.stack([w_emb[:D], w_emb[D:]], axis=1).astype(np.float32)

    in_maps = []
    for m in range(8):
        in_maps.append({
            "hT": hT_np, "embT": embT_np,
            "W_lin": W_lin, "W_linT": W_lin.T.copy(),
            "wfc": wfc_np, "wemb": wemb_np,
            "iota_in": iota_np, "ident_in": ident_np,
            "idx_in": per_core[m]["idx"],
            "dstloc_in": per_core[m]["dstloc"],
            "nid_in": per_core[m]["nid"],
        })

    trace = os.environ.get("GAT_TRACE", "0") == "1"
    res = bass_utils.run_bass_kernel_spmd(
        nc, in_maps, core_ids=list(range(8)), trace=trace)
    LAST_EXEC_NS = res.exec_time_ns

    out = np.zeros((NNODES_PAD, P), np.float32)
    for m in range(8):
        t0 = m * TSLOTS
        nt = min(TSLOTS, NTILES - t0)
        if nt <= 0:
            continue
        out[t0 * P:(t0 + nt) * P] = res.results[m]["out"][: nt * P]
    return out[:N]
